# revision 4
# baseline (speedup 1.0000x reference)
# Trainium2 Bass kernel for DigitConvolutionalModel:
#   out = relu(conv3x3(x) @ w1 + b1) @ w2 + b2
# The 3x3 valid conv and the (676,200) matmul are both linear in x, so they
# fold (host-side, float64) into a single (784,200) matrix W_eff.  Each of the
# 8 cores gets 8192 rows of x, shipped pre-transposed as xT (784, 8192) so the
# contraction dim sits on SBUF partitions and every DMA is contiguous.
# On-chip per core:
#   hiddenT = relu(W_eff.T @ xT + b1)   (PE matmuls, PSUM-accumulated over K)
#   outT    = w2.T @ hiddenT + b2
# outT (10, 8192) is DMA'd out and transposed back on the host.
import os

import numpy as np

_B = 65536
_IMG = 784  # 28*28
_HPX = 28
_KW = 3
_OUT = 26
_HID = 200
_NCLS = 10
_NCORES = 8
_ROWS = _B // _NCORES  # 8192
_N = 512  # matmul moving free dim (one PSUM bank of fp32)
_DMA_CHUNK = 2048  # columns per input DMA (8KB per partition)
_KCH = [128, 128, 128, 128, 128, 128, 16]  # 784 = 6*128 + 16
_HCH = [(0, 128), (128, 72)]  # 200 = 128 + 72

# matmul dtype mode: "f32r" (fp32 storage, reduced-precision full-rate matmul),
# "bf16" (half DMA traffic), "f32" (exact, 4 cycles/row)
_MODE = os.environ.get("KMODE", "f32r")

_CACHE = {}

# set after each run (for the test harness)
LAST_EXEC_NS = None


def _np_in_dtype():
    if _MODE == "bf16":
        import ml_dtypes

        return np.dtype(ml_dtypes.bfloat16)
    return np.dtype(np.float32)


def _build():
    import concourse.mybir as mybir
    from concourse import bacc
    from concourse.tile import TileContext

    if _MODE == "bf16":
        DT = mybir.dt.bfloat16
    elif _MODE == "f32":
        DT = mybir.dt.float32
    else:
        DT = mybir.dt.float32r
    F32 = mybir.dt.float32
    Relu = mybir.ActivationFunctionType.Relu
    Ident = mybir.ActivationFunctionType.Identity

    nc = bacc.Bacc()
    xT = nc.declare_dram_parameter("xT", [_IMG, _ROWS], DT, isOutput=False)
    weff = nc.declare_dram_parameter("weff", [_IMG, _HID], DT, isOutput=False)
    w2 = nc.declare_dram_parameter("w2", [_HID, _NCLS], DT, isOutput=False)
    b1 = nc.declare_dram_parameter("b1", [_HID, 1], F32, isOutput=False)
    b2 = nc.declare_dram_parameter("b2", [_NCLS, 1], F32, isOutput=False)
    outT = nc.declare_dram_parameter("outT", [_NCLS, _ROWS], F32, isOutput=True)

    with TileContext(nc) as tc:
        with (
            tc.tile_pool(name="const", bufs=1) as cpool,
            tc.tile_pool(name="xin", bufs=2) as xpool,
            tc.tile_pool(name="hid", bufs=3) as hpool,
            tc.tile_pool(name="osb", bufs=3) as opool,
            tc.tile_pool(name="ps1", bufs=4, space="PSUM") as ps1pool,
            tc.tile_pool(name="ps2", bufs=2, space="PSUM") as ps2pool,
        ):
            weff_sb = []
            k0 = 0
            for ki, kc in enumerate(_KCH):
                wt = cpool.tile([kc, _HID], DT, name=f"weff{ki}", tag=f"weff{ki}")
                nc.sync.dma_start(out=wt[:, :], in_=weff[k0 : k0 + kc, :])
                weff_sb.append(wt)
                k0 += kc
            w2_sb = []
            b1_sb = []
            for hi, (h0, hc) in enumerate(_HCH):
                w2t = cpool.tile([hc, _NCLS], DT, name=f"w2_{hi}", tag=f"w2_{hi}")
                nc.sync.dma_start(out=w2t[:, :], in_=w2[h0 : h0 + hc, :])
                w2_sb.append(w2t)
                b1t = cpool.tile([hc, 1], F32, name=f"b1_{hi}", tag=f"b1_{hi}")
                nc.sync.dma_start(out=b1t[:, :], in_=b1[h0 : h0 + hc, :])
                b1_sb.append(b1t)
            b2_sb = cpool.tile([_NCLS, 1], F32, name="b2sb", tag="b2sb")
            nc.sync.dma_start(out=b2_sb[:, :], in_=b2[:, :])

            for ci in range(_ROWS // _DMA_CHUNK):
                xt = []
                k0 = 0
                for ki, kc in enumerate(_KCH):
                    t = xpool.tile([kc, _DMA_CHUNK], DT, name=f"xt{ki}", tag=f"xt{ki}")
                    nc.sync.dma_start(
                        out=t[:, :],
                        in_=xT[k0 : k0 + kc, ci * _DMA_CHUNK : (ci + 1) * _DMA_CHUNK],
                    )
                    xt.append(t)
                    k0 += kc
                for gi in range(_DMA_CHUNK // _N):
                    col = gi * _N
                    hsb = []
                    for hi, (h0, hc) in enumerate(_HCH):
                        ps1 = ps1pool.tile([hc, _N], F32, name=f"ps1_{hi}", tag="ps1")
                        for ki in range(len(_KCH)):
                            nc.tensor.matmul(
                                ps1[:, :],
                                lhsT=weff_sb[ki][:, h0 : h0 + hc],
                                rhs=xt[ki][:, col : col + _N],
                                start=(ki == 0),
                                stop=(ki == len(_KCH) - 1),
                            )
                        h = hpool.tile([hc, _N], DT, name=f"h{hi}", tag=f"h{hi}")
                        nc.scalar.activation(
                            h[:, :], ps1[:, :], Relu, bias=b1_sb[hi][:, :], scale=1.0
                        )
                        hsb.append(h)
                    ps2 = ps2pool.tile([_NCLS, _N], F32, name="ps2", tag="ps2")
                    for hi in range(len(_HCH)):
                        nc.tensor.matmul(
                            ps2[:, :],
                            lhsT=w2_sb[hi][:, :],
                            rhs=hsb[hi][:, :],
                            start=(hi == 0),
                            stop=(hi == len(_HCH) - 1),
                        )
                    osb = opool.tile([_NCLS, _N], F32, name="osb", tag="osb")
                    nc.scalar.activation(
                        osb[:, :], ps2[:, :], Ident, bias=b2_sb[:, :], scale=1.0
                    )
                    acol = ci * _DMA_CHUNK + col
                    nc.sync.dma_start(out=outT[:, acol : acol + _N], in_=osb[:, :])
    nc.finalize()
    return nc


def _get_nc():
    if _MODE not in _CACHE:
        _CACHE[_MODE] = _build()
    return _CACHE[_MODE]


def _fold_weights(conv_w, w1):
    """Fold the 3x3 valid conv into w1: returns (784, 200) float32."""
    w1r = np.asarray(w1, np.float64).reshape(_OUT, _OUT, _HID)
    cw = np.asarray(conv_w, np.float64)
    weff = np.zeros((_HPX, _HPX, _HID), np.float64)
    for ki in range(_KW):
        for kj in range(_KW):
            weff[ki : ki + _OUT, kj : kj + _OUT, :] += cw[ki, kj] * w1r
    return weff.reshape(_IMG, _HID).astype(np.float32)


def kernel(**inputs):
    global LAST_EXEC_NS
    from concourse.bass_utils import run_bass_kernel_spmd

    x = np.asarray(inputs["x"], np.float32)
    conv_w = inputs["conv_w"]
    w1 = inputs["w1"]
    b1 = np.asarray(inputs["b1"], np.float32).reshape(_HID, 1)
    w2 = np.asarray(inputs["w2"], np.float32)
    b2 = np.asarray(inputs["b2"], np.float32).reshape(_NCLS, 1)

    ind = _np_in_dtype()
    weff = _fold_weights(conv_w, w1).astype(ind)
    w2c = np.ascontiguousarray(w2.astype(ind))

    in_maps = []
    for c in range(_NCORES):
        xs = x[c * _ROWS : (c + 1) * _ROWS]
        in_maps.append(
            {
                "xT": np.ascontiguousarray(xs.T.astype(ind)),
                "weff": weff,
                "w2": w2c,
                "b1": b1,
                "b2": b2,
            }
        )

    nc = _get_nc()
    res = run_bass_kernel_spmd(nc, in_maps, list(range(_NCORES)))
    LAST_EXEC_NS = res.exec_time_ns

    out = np.empty((_B, _NCLS), np.float32)
    for c in range(_NCORES):
        out[c * _ROWS : (c + 1) * _ROWS, :] = res.results[c]["outT"].T
    return out
